# revision 19
# baseline (speedup 1.0000x reference)
"""AttentivePooling Trainium2 kernel (streaming, bf16 score path).

Reference semantics (h_all: [T, B, D] f32, xin unused):
    h_last = h_all[-1]                       # [B, D]
    a[b, t] = <h_all[t, b, :], h_last[b, :]> / sqrt(D)
    r = relu(a)
    w = r / (sum_t r + 1e-9)
    out[b, d] = sum_t w[b, t] * h_all[t, b, d]

Because normalization happens after the relu, out = num / (Z + eps) with
num[b] = sum_t relu(a[b,t]) h[t,b] and Z[b] = sum_t relu(a[b,t]) -- both
accumulate chunk-by-chunk, so one streaming pass over h suffices.

Strategy: data-parallel over B across 8 cores (8 batches/core).  Per core,
stream 16 T-chunks of [128(t), 8(b)*512(d)]:
  - each chunk is ONE fully contiguous 2MB HWDGE DMA (128 rows x 16KB)
    issued from the Sync engine so loads never wait on compute engines;
    6 chunk buffers keep the DMA queues saturated.
  - score multiplies read h as bf16 via a stride-2 view of the f32 data
    (the top 2 bytes of an f32 are its truncated bf16), halving SBUF read
    traffic; h_last is pre-converted to packed bf16 with the 1/sqrt(D)
    scale folded in, one private copy for DVE and one for GPSIMD (a
    shared copy measurably slows both engines).
  - multiplies: 5 batches on DVE, 3 on GPSIMD; reductions of the bf16
    products: 6 on ACT (accum_out), 2 on DVE (tensor_reduce).
  - one ACT relu per chunk produces w [128, 8] (f32r); a DVE add
    accumulates w into wacc for the Z computation.
  - PE accumulates num[b] into 8 per-batch [1, 512] PSUM banks (full
    fp32 h as rhs, f32r fast path).
  - epilogue: GPSIMD partition_all_reduce gives Z; DVE computes
    1/(Z+eps); ACT scales each pooled row into quadrant-aligned result
    rows; two strided 8KB stores.
"""

import numpy as np
from contextlib import ExitStack

import concourse.bass as bass
import concourse.tile as tile
from concourse import bacc, mybir
from concourse.bass_utils import run_bass_kernel_spmd

T, B, D = 2048, 64, 512
NCORES = 8
BPC = B // NCORES  # batches per core
P = 128
TC = T // P  # 16 T-chunks
BD = BPC * D  # 4096
SCALE = float(1.0 / np.sqrt(np.float32(D)))
NV = 5  # batches 0..NV-1 multiplied on DVE (packed bf16); rest on GPSIMD
NRV = 3  # batches 0..NRV-1 reduced on DVE in one batched instr; rest on ACT
NBUF = 6  # chunk buffers in flight

_nc_cache = None


def _build():
    global _nc_cache
    if _nc_cache is not None:
        return _nc_cache
    nc = bacc.Bacc("TRN2", debug=False, target_bir_lowering=False, num_devices=NCORES)
    h = nc.dram_tensor("h", [T, BPC, D], mybir.dt.float32r, kind="ExternalInput")
    out = nc.dram_tensor("out", [BPC, D], mybir.dt.float32, kind="ExternalOutput")
    h_ap = h.ap()
    out_ap = out.ap()
    f32 = mybir.dt.float32
    f32r = mybir.dt.float32r
    bf16 = mybir.dt.bfloat16

    with tile.TileContext(nc) as tc:
        with ExitStack() as ctx:
            hpool = ctx.enter_context(tc.tile_pool(name="h", bufs=NBUF))
            tmpp = ctx.enter_context(tc.tile_pool(name="tmp", bufs=4))
            tmpg = ctx.enter_context(tc.tile_pool(name="tmpg", bufs=3))
            scwp = ctx.enter_context(tc.tile_pool(name="scw", bufs=3))
            constp = ctx.enter_context(tc.tile_pool(name="const", bufs=1))
            psp = ctx.enter_context(tc.tile_pool(name="ps", bufs=1, space="PSUM"))

            # per-partition running sum of relu'd scores (one col per batch);
            # reduced across partitions once at the end on GPSIMD
            wacc = constp.tile([P, BPC], f32, name="wacc")
            nc.vector.memset(wacc[:], 0.0)

            # h_last broadcast to all partitions straight from DRAM, then
            # converted to packed bf16 with SCALE folded in -- one private
            # copy per multiplying engine
            hl_f32 = constp.tile([P, BD], f32, name="hl_f32")
            src_bc = (
                h_ap[T - 1 : T, :, :]
                .bitcast(f32)
                .rearrange("p b d -> p (b d)")
                .broadcast_to([P, BD])
            )
            nc.sync.dma_start(hl_f32[:], src_bc)
            hl_v = constp.tile([P, BD], bf16, name="hl_v")
            nc.vector.tensor_scalar_mul(hl_v[:], hl_f32[:], SCALE)
            hl_g = constp.tile([P, BD], bf16, name="hl_g")
            nc.scalar.activation(
                hl_g[:], hl_f32[:], mybir.ActivationFunctionType.Copy, scale=SCALE
            )

            # persistent accumulators: one PSUM bank per batch (matmul
            # outputs with K=128 must start at partition 0)
            pouts = [psp.tile([1, D], f32, name=f"pout{b}") for b in range(BPC)]

            # ACT writes need quadrant-aligned partition offsets: result
            # rows live at partitions {0,32,64,96} of two tiles.
            res = [constp.tile([P, D], f32, name=f"res{i}") for i in range(2)]
            zeps = constp.tile([1, BPC], f32, name="zeps")
            zrec = constp.tile([1, BPC], f32, name="zrec")

            hc_tiles = {}

            def load(c):
                t = hpool.tile([P, BPC, D], f32r, tag="hc", name="h_sb")
                nc.sync.dma_start(t[:], h_ap[c * P : (c + 1) * P, :, :])
                hc_tiles[c] = t

            for c in range(min(NBUF - 1, TC)):
                load(c)

            for c in range(TC):
                hc = hc_tiles.pop(c)
                scr = scwp.tile([P, BPC], f32, tag="scr")
                w = scwp.tile([P, BPC], f32r, tag="w")

                # pack DVE's share of h to bf16 in one 2x-mode copy so its
                # multiplies run in the fast all-packed-bf16 mode
                hc_bf = tmpp.tile([P, NV, D], bf16, tag="hcbf")
                nc.vector.tensor_copy(
                    hc_bf[:], hc[:, 0:NV, :].bitcast(bf16)[:, :, 1::2]
                )

                # DVE-reduced products share one tile -> one batched reduce
                prod_v = tmpp.tile([P, NRV, D], bf16, tag="tv")
                act_prods = {}
                for b in range(BPC):
                    if b < NV:
                        if b < NRV:
                            prod = prod_v[:, b, :]
                        else:
                            pt = tmpp.tile([P, D], bf16, tag="tv1")
                            act_prods[b] = pt
                            prod = pt[:]
                        nc.vector.tensor_tensor(
                            prod, hc_bf[:, b, :], hl_v[:, b * D : (b + 1) * D],
                            mybir.AluOpType.mult,
                        )
                    else:
                        pt = tmpg.tile([P, D], bf16, tag="tg")
                        act_prods[b] = pt
                        nc.gpsimd.tensor_tensor(
                            pt[:],
                            hc[:, b, :].bitcast(bf16)[:, 1::2],
                            hl_g[:, b * D : (b + 1) * D],
                            mybir.AluOpType.mult,
                        )

                nc.vector.tensor_reduce(
                    scr[:, 0:NRV],
                    prod_v[:],
                    mybir.AxisListType.X,
                    mybir.AluOpType.add,
                )
                for b, pt in sorted(act_prods.items()):
                    nc.scalar.activation(
                        pt[:],
                        pt[:],
                        mybir.ActivationFunctionType.Copy,
                        accum_out=scr[:, b : b + 1],
                    )

                nc.scalar.activation(w[:], scr[:], mybir.ActivationFunctionType.Relu)
                nc.gpsimd.tensor_tensor(
                    wacc[:], wacc[:], w[:].bitcast(f32), mybir.AluOpType.add
                )

                if c + NBUF - 1 < TC:
                    load(c + NBUF - 1)

                for b in range(BPC):
                    nc.tensor.matmul(
                        pouts[b][:],
                        w[:, b : b + 1],
                        hc[:, b, :],
                        start=(c == 0),
                        stop=(c == TC - 1),
                    )

            zred = constp.tile([P, BPC], f32, name="zred")
            nc.gpsimd.partition_all_reduce(
                zred[:], wacc[:], channels=P, reduce_op=bass.bass_isa.ReduceOp.add
            )
            nc.vector.tensor_scalar_add(zeps[:], zred[0:1, :], 1e-9)
            nc.vector.reciprocal(zrec[:], zeps[:])
            for b in range(BPC):
                rt, rrow = res[b // 4], (b % 4) * 32
                nc.scalar.mul(
                    rt[rrow : rrow + 1, :], pouts[b][:], zrec[0:1, b : b + 1]
                )
            nc.sync.dma_start(out_ap[0:4, :], res[0][0:P:32, :])
            nc.sync.dma_start(out_ap[4:8, :], res[1][0:P:32, :])

    nc.finalize()
    _nc_cache = nc
    return nc


def _run(h_all: np.ndarray, trace: bool = False):
    nc = _build()
    h_all = np.ascontiguousarray(np.asarray(h_all), dtype=np.float32)
    assert h_all.shape == (T, B, D)
    in_maps = [
        {"h": np.ascontiguousarray(h_all[:, c * BPC : (c + 1) * BPC, :])}
        for c in range(NCORES)
    ]
    r = run_bass_kernel_spmd(nc, in_maps, list(range(NCORES)), trace=trace)
    out = np.concatenate([r.results[c]["out"] for c in range(NCORES)], axis=0)
    return out, r


def kernel(h_all: np.ndarray, xin: np.ndarray | None = None) -> np.ndarray:
    out, _ = _run(h_all)
    return out


# revision 23
# speedup vs baseline: 1.3929x; 1.3929x over previous
"""AttentivePooling Trainium2 kernel (streaming, bf16 score path).

Reference semantics (h_all: [T, B, D] f32, xin unused):
    h_last = h_all[-1]                       # [B, D]
    a[b, t] = <h_all[t, b, :], h_last[b, :]> / sqrt(D)
    r = relu(a)
    w = r / (sum_t r + 1e-9)
    out[b, d] = sum_t w[b, t] * h_all[t, b, d]

Because normalization happens after the relu, out = num / (Z + eps) with
num[b] = sum_t relu(a[b,t]) h[t,b] and Z[b] = sum_t relu(a[b,t]) -- both
accumulate chunk-by-chunk, so one streaming pass over h suffices.

Strategy: data-parallel over B across 8 cores (8 batches/core).  Per core,
stream 16 T-chunks of [128(t), 8(b)*512(d)]:
  - each chunk is ONE fully contiguous 2MB HWDGE DMA (128 rows x 16KB)
    issued from the Sync engine so loads never wait on compute engines;
    6 chunk buffers keep the DMA queues saturated.
  - score multiplies read h as bf16 via a stride-2 view of the f32 data
    (the top 2 bytes of an f32 are its truncated bf16), halving SBUF read
    traffic; h_last is pre-converted to packed bf16 with the 1/sqrt(D)
    scale folded in, one private copy for DVE and one for GPSIMD (a
    shared copy measurably slows both engines).
  - multiplies: 5 batches on DVE, 3 on GPSIMD; reductions of the bf16
    products: 6 on ACT (accum_out), 2 on DVE (tensor_reduce).
  - one ACT relu per chunk produces w [128, 8] (f32r); a DVE add
    accumulates w into wacc for the Z computation.
  - PE accumulates num[b] into 8 per-batch [1, 512] PSUM banks (full
    fp32 h as rhs, f32r fast path).
  - epilogue: GPSIMD partition_all_reduce gives Z; DVE computes
    1/(Z+eps); ACT scales each pooled row into quadrant-aligned result
    rows; two strided 8KB stores.
"""

import numpy as np
from contextlib import ExitStack

import concourse.bass as bass
import concourse.tile as tile
from concourse import bacc, mybir
from concourse.bass_utils import run_bass_kernel_spmd

T, B, D = 2048, 64, 512
NCORES = 8
BPC = B // NCORES  # batches per core
P = 128
TC = T // P  # 16 T-chunks
BD = BPC * D  # 4096
SCALE = float(1.0 / np.sqrt(np.float32(D)))
NV = 4  # batches 0..NV-1 on DVE (fused f32 multiply+reduce); rest on GPSIMD
NBUF = 6  # chunk buffers in flight

_nc_cache = None


def _build():
    global _nc_cache
    if _nc_cache is not None:
        return _nc_cache
    nc = bacc.Bacc("TRN2", debug=False, target_bir_lowering=False, num_devices=NCORES)
    h = nc.dram_tensor("h", [T, BPC, D], mybir.dt.float32r, kind="ExternalInput")
    out = nc.dram_tensor("out", [BPC, D], mybir.dt.float32, kind="ExternalOutput")
    h_ap = h.ap()
    out_ap = out.ap()
    f32 = mybir.dt.float32
    f32r = mybir.dt.float32r
    bf16 = mybir.dt.bfloat16

    with tile.TileContext(nc) as tc:
        with ExitStack() as ctx:
            hpool = ctx.enter_context(tc.tile_pool(name="h", bufs=NBUF))
            tmpp = ctx.enter_context(tc.tile_pool(name="tmp", bufs=3))
            tmpg = ctx.enter_context(tc.tile_pool(name="tmpg", bufs=6))
            scwp = ctx.enter_context(tc.tile_pool(name="scw", bufs=3))
            constp = ctx.enter_context(tc.tile_pool(name="const", bufs=1))
            psp = ctx.enter_context(tc.tile_pool(name="ps", bufs=1, space="PSUM"))

            # per-partition running sum of relu'd scores (one col per batch);
            # reduced across partitions once at the end on GPSIMD
            wacc = constp.tile([P, BPC], f32, name="wacc")
            nc.vector.memset(wacc[:], 0.0)

            # h_last broadcast to all partitions straight from DRAM, then
            # converted to packed bf16 with SCALE folded in -- one private
            # copy per multiplying engine
            hl_f32 = constp.tile([P, BD], f32, name="hl_f32")
            src_bc = (
                h_ap[T - 1 : T, :, :]
                .bitcast(f32)
                .rearrange("p b d -> p (b d)")
                .broadcast_to([P, BD])
            )
            nc.sync.dma_start(hl_f32[:], src_bc)
            hl_g = constp.tile([P, BD], bf16, name="hl_g")
            nc.scalar.activation(
                hl_g[:], hl_f32[:], mybir.ActivationFunctionType.Copy, scale=SCALE
            )

            # persistent accumulators: one PSUM bank per batch (matmul
            # outputs with K=128 must start at partition 0)
            pouts = [psp.tile([1, D], f32, name=f"pout{b}") for b in range(BPC)]

            # ACT writes need quadrant-aligned partition offsets: result
            # rows live at partitions {0,32,64,96} of two tiles.
            res = [constp.tile([P, D], f32, name=f"res{i}") for i in range(2)]
            zeps = constp.tile([1, BPC], f32, name="zeps")
            zrec = constp.tile([1, BPC], f32, name="zrec")

            hc_tiles = {}

            def load(c):
                t = hpool.tile([P, BPC, D], f32r, tag="hc", name="h_sb")
                nc.sync.dma_start(t[:], h_ap[c * P : (c + 1) * P, :, :])
                hc_tiles[c] = t

            for c in range(min(NBUF - 1, TC)):
                load(c)

            for c in range(TC):
                hc = hc_tiles.pop(c)
                scr = scwp.tile([P, BPC], f32, tag="scr")
                w = scwp.tile([P, BPC], f32r, tag="w")

                gp_prods = {}
                for b in range(BPC):
                    if b < NV:
                        # fused fp32 multiply + row-sum in one DVE pass
                        tmp = tmpp.tile([P, D], f32, tag="tv")
                        nc.vector.scalar_tensor_tensor(
                            tmp[:],
                            hc[:, b, :].bitcast(f32),
                            SCALE,
                            hl_f32[:, b * D : (b + 1) * D],
                            mybir.AluOpType.mult,
                            mybir.AluOpType.mult,
                            accum_out=scr[:, b : b + 1],
                        )
                    else:
                        pt = tmpg.tile([P, D], bf16, tag="tg")
                        gp_prods[b] = pt
                        nc.gpsimd.tensor_tensor(
                            pt[:],
                            hc[:, b, :].bitcast(bf16)[:, 1::2],
                            hl_g[:, b * D : (b + 1) * D],
                            mybir.AluOpType.mult,
                        )

                for b, pt in sorted(gp_prods.items()):
                    nc.scalar.activation(
                        pt[:],
                        pt[:],
                        mybir.ActivationFunctionType.Copy,
                        accum_out=scr[:, b : b + 1],
                    )

                nc.scalar.activation(w[:], scr[:], mybir.ActivationFunctionType.Relu)
                nc.vector.tensor_tensor(
                    wacc[:], wacc[:], w[:].bitcast(f32), mybir.AluOpType.add
                )

                if c + NBUF - 1 < TC:
                    load(c + NBUF - 1)

                for b in range(BPC):
                    nc.tensor.matmul(
                        pouts[b][:],
                        w[:, b : b + 1],
                        hc[:, b, :],
                        start=(c == 0),
                        stop=(c == TC - 1),
                    )

            zred = constp.tile([P, BPC], f32, name="zred")
            nc.gpsimd.partition_all_reduce(
                zred[:], wacc[:], channels=P, reduce_op=bass.bass_isa.ReduceOp.add
            )
            nc.vector.tensor_scalar_add(zeps[:], zred[0:1, :], 1e-9)
            nc.vector.reciprocal(zrec[:], zeps[:])
            for b in range(BPC):
                rt, rrow = res[b // 4], (b % 4) * 32
                nc.scalar.mul(
                    rt[rrow : rrow + 1, :], pouts[b][:], zrec[0:1, b : b + 1]
                )
            nc.sync.dma_start(out_ap[0:4, :], res[0][0:P:32, :])
            nc.sync.dma_start(out_ap[4:8, :], res[1][0:P:32, :])

    nc.finalize()
    _nc_cache = nc
    return nc


def _run(h_all: np.ndarray, trace: bool = False):
    nc = _build()
    h_all = np.ascontiguousarray(np.asarray(h_all), dtype=np.float32)
    assert h_all.shape == (T, B, D)
    in_maps = [
        {"h": np.ascontiguousarray(h_all[:, c * BPC : (c + 1) * BPC, :])}
        for c in range(NCORES)
    ]
    r = run_bass_kernel_spmd(nc, in_maps, list(range(NCORES)), trace=trace)
    out = np.concatenate([r.results[c]["out"] for c in range(NCORES)], axis=0)
    return out, r


def kernel(h_all: np.ndarray, xin: np.ndarray | None = None) -> np.ndarray:
    out, _ = _run(h_all)
    return out


# revision 24
# speedup vs baseline: 1.4038x; 1.0078x over previous
"""AttentivePooling Trainium2 kernel (streaming, mixed fused-DVE/GPSIMD).

Reference semantics (h_all: [T, B, D] f32, xin unused):
    h_last = h_all[-1]                       # [B, D]
    a[b, t] = <h_all[t, b, :], h_last[b, :]> / sqrt(D)
    r = relu(a)
    w = r / (sum_t r + 1e-9)
    out[b, d] = sum_t w[b, t] * h_all[t, b, d]

Because normalization happens after the relu, out = num / (Z + eps) with
num[b] = sum_t relu(a[b,t]) h[t,b] and Z[b] = sum_t relu(a[b,t]) -- both
accumulate chunk-by-chunk, so one streaming pass over h suffices.

Strategy: data-parallel over B across 8 cores (8 batches/core).  Per core,
stream 16 T-chunks of [128(t), 8(b)*512(d)]:
  - each chunk is ONE fully contiguous 2MB HWDGE DMA (128 rows x 16KB)
    issued from the Sync engine so loads never wait on compute engines
    (chunk 0 and the h_last broadcast are split in halves and interleaved
    so compute starts ~7us in).
  - scores for batches 0-3: fused fp32 multiply+row-sum in one DVE
    scalar_tensor_tensor (with the 1/sqrt(D) scale as the immediate);
    batches 4-7: GPSIMD bf16 multiply (stride-2 high-half view of the
    f32 data) + ACT copy reduce with accum_out.  Private h_last copies
    per engine (a shared tile measurably slows both).
  - one ACT relu per chunk writes w into a persistent wall[128,16,8]
    tile; Z comes from one strided reduce + partition_all_reduce at the
    end (no per-chunk accumulate).
  - PE accumulates num[b] into 8 per-batch [1, 512] PSUM banks.
  - epilogue: 1/(Z+eps) on DVE; final scaling split DVE/ACT; two
    strided 8KB stores.
"""

import numpy as np
from contextlib import ExitStack

import concourse.bass as bass
import concourse.tile as tile
from concourse import bacc, mybir
from concourse.bass_utils import run_bass_kernel_spmd

T, B, D = 2048, 64, 512
NCORES = 8
BPC = B // NCORES  # batches per core
P = 128
TC = T // P  # 16 T-chunks
BD = BPC * D  # 4096
SCALE = float(1.0 / np.sqrt(np.float32(D)))
NV = 4  # batches 0..NV-1 on DVE (fused f32 multiply+reduce); rest on GPSIMD
NBUF = 6  # chunk buffers in flight

_nc_cache = None


def _build():
    global _nc_cache
    if _nc_cache is not None:
        return _nc_cache
    nc = bacc.Bacc("TRN2", debug=False, target_bir_lowering=False, num_devices=NCORES)
    h = nc.dram_tensor("h", [T, BPC, D], mybir.dt.float32r, kind="ExternalInput")
    out = nc.dram_tensor("out", [BPC, D], mybir.dt.float32, kind="ExternalOutput")
    h_ap = h.ap()
    out_ap = out.ap()
    f32 = mybir.dt.float32
    f32r = mybir.dt.float32r
    bf16 = mybir.dt.bfloat16

    with tile.TileContext(nc) as tc:
        with ExitStack() as ctx:
            hpool = ctx.enter_context(tc.tile_pool(name="h", bufs=NBUF))
            tmpp = ctx.enter_context(tc.tile_pool(name="tmp", bufs=3))
            tmpg = ctx.enter_context(tc.tile_pool(name="tmpg", bufs=6))
            constp = ctx.enter_context(tc.tile_pool(name="const", bufs=1))
            psp = ctx.enter_context(tc.tile_pool(name="ps", bufs=1, space="PSUM"))

            # h_last broadcast straight from DRAM in two halves so the DVE
            # half is ready early; GPSIMD's half is converted to bf16 (with
            # SCALE folded) by ACT
            hl_f32 = constp.tile([P, NV * D], f32, name="hl_f32")
            hl_gsrc = constp.tile([P, BD - NV * D], f32, name="hl_gsrc")

            def hl_bcast_ap(b0, b1):
                return (
                    h_ap[T - 1 : T, b0:b1, :]
                    .bitcast(f32)
                    .rearrange("p b d -> p (b d)")
                    .broadcast_to([P, (b1 - b0) * D])
                )

            hc_tiles = {}

            def load(c, split=False):
                t = hpool.tile([P, BPC, D], f32r, tag="hc", name="h_sb")
                if split:
                    nc.sync.dma_start(t[:, 0:NV, :], h_ap[c * P : (c + 1) * P, 0:NV, :])
                else:
                    nc.sync.dma_start(t[:], h_ap[c * P : (c + 1) * P, :, :])
                hc_tiles[c] = t

            # startup order: DVE's inputs first, then GPSIMD's
            nc.sync.dma_start(hl_f32[:], hl_bcast_ap(0, NV))
            load(0, split=True)
            nc.sync.dma_start(hl_gsrc[:], hl_bcast_ap(NV, BPC))
            nc.sync.dma_start(
                hc_tiles[0][:, NV:BPC, :], h_ap[0:P, NV:BPC, :]
            )
            for c in range(1, min(NBUF - 1, TC)):
                load(c)

            hl_g = constp.tile([P, BD - NV * D], bf16, name="hl_g")
            nc.scalar.activation(
                hl_g[:], hl_gsrc[:], mybir.ActivationFunctionType.Copy, scale=SCALE
            )

            # persistent accumulators: one PSUM bank per batch (matmul
            # outputs with K=128 must start at partition 0)
            pouts = [psp.tile([1, D], f32, name=f"pout{b}") for b in range(BPC)]

            # all chunks' scores and relu'd weights live in two small
            # persistent tiles; Z is reduced from wall once at the end
            scrall = constp.tile([P, TC, BPC], f32, name="scrall")
            wall = constp.tile([P, TC, BPC], f32r, name="wall")

            # ACT/DVE writes need quadrant-aligned partition offsets:
            # result rows live at partitions {0,32,64,96} of two tiles.
            res = [constp.tile([P, D], f32, name=f"res{i}") for i in range(2)]
            zsum = constp.tile([P, BPC], f32, name="zsum")
            zred = constp.tile([P, BPC], f32, name="zred")
            zeps = constp.tile([1, BPC], f32, name="zeps")
            zrec = constp.tile([1, BPC], f32, name="zrec")

            for c in range(TC):
                hc = hc_tiles.pop(c)
                scr = scrall[:, c, :]
                w = wall[:, c, :]

                gp_prods = {}
                for b in range(BPC):
                    if b < NV:
                        # fused fp32 multiply + row-sum in one DVE pass
                        tmp = tmpp.tile([P, D], f32, tag="tv")
                        nc.vector.scalar_tensor_tensor(
                            tmp[:],
                            hc[:, b, :].bitcast(f32),
                            SCALE,
                            hl_f32[:, b * D : (b + 1) * D],
                            mybir.AluOpType.mult,
                            mybir.AluOpType.mult,
                            accum_out=scr[:, b : b + 1],
                        )
                    else:
                        pt = tmpg.tile([P, D], bf16, tag="tg")
                        gp_prods[b] = pt
                        nc.gpsimd.tensor_tensor(
                            pt[:],
                            hc[:, b, :].bitcast(bf16)[:, 1::2],
                            hl_g[:, (b - NV) * D : (b - NV + 1) * D],
                            mybir.AluOpType.mult,
                        )

                for b, pt in sorted(gp_prods.items()):
                    nc.scalar.activation(
                        pt[:],
                        pt[:],
                        mybir.ActivationFunctionType.Copy,
                        accum_out=scr[:, b : b + 1],
                    )

                nc.scalar.activation(w, scr, mybir.ActivationFunctionType.Relu)

                if c + NBUF - 1 < TC:
                    load(c + NBUF - 1)

                for b in range(BPC):
                    nc.tensor.matmul(
                        pouts[b][:],
                        w[:, b : b + 1],
                        hc[:, b, :],
                        start=(c == 0),
                        stop=(c == TC - 1),
                    )

            # Z[b] = sum over chunks and partitions of relu'd scores
            nc.vector.tensor_reduce(
                zsum[:],
                wall[:].bitcast(f32).rearrange("p c b -> p b c"),
                mybir.AxisListType.X,
                mybir.AluOpType.add,
            )
            nc.gpsimd.partition_all_reduce(
                zred[:], zsum[:], channels=P, reduce_op=bass.bass_isa.ReduceOp.add
            )
            nc.vector.tensor_scalar_add(zeps[:], zred[0:1, :], 1e-9)
            nc.vector.reciprocal(zrec[:], zeps[:])
            for b in range(BPC):
                rt, rrow = res[b // 4], (b % 4) * 32
                if b % 2 == 0:
                    nc.vector.tensor_scalar_mul(
                        rt[rrow : rrow + 1, :], pouts[b][:], zrec[0:1, b : b + 1]
                    )
                else:
                    nc.scalar.mul(
                        rt[rrow : rrow + 1, :], pouts[b][:], zrec[0:1, b : b + 1]
                    )
            nc.sync.dma_start(out_ap[0:4, :], res[0][0:P:32, :])
            nc.sync.dma_start(out_ap[4:8, :], res[1][0:P:32, :])

    nc.finalize()
    _nc_cache = nc
    return nc


def _run(h_all: np.ndarray, trace: bool = False):
    nc = _build()
    h_all = np.ascontiguousarray(np.asarray(h_all), dtype=np.float32)
    assert h_all.shape == (T, B, D)
    in_maps = [
        {"h": np.ascontiguousarray(h_all[:, c * BPC : (c + 1) * BPC, :])}
        for c in range(NCORES)
    ]
    r = run_bass_kernel_spmd(nc, in_maps, list(range(NCORES)), trace=trace)
    out = np.concatenate([r.results[c]["out"] for c in range(NCORES)], axis=0)
    return out, r


def kernel(h_all: np.ndarray, xin: np.ndarray | None = None) -> np.ndarray:
    out, _ = _run(h_all)
    return out


# revision 26
# speedup vs baseline: 1.4145x; 1.0076x over previous
"""AttentivePooling Trainium2 kernel (streaming, mixed fused-DVE/GPSIMD).

Reference semantics (h_all: [T, B, D] f32, xin unused):
    h_last = h_all[-1]                       # [B, D]
    a[b, t] = <h_all[t, b, :], h_last[b, :]> / sqrt(D)
    r = relu(a)
    w = r / (sum_t r + 1e-9)
    out[b, d] = sum_t w[b, t] * h_all[t, b, d]

Because normalization happens after the relu, out = num / (Z + eps) with
num[b] = sum_t relu(a[b,t]) h[t,b] and Z[b] = sum_t relu(a[b,t]) -- both
accumulate chunk-by-chunk, so one streaming pass over h suffices.

Strategy: data-parallel over B across 8 cores (8 batches/core).  Per core,
stream 16 T-chunks of [128(t), 8(b)*512(d)]:
  - each chunk is ONE fully contiguous 2MB HWDGE DMA (128 rows x 16KB)
    issued from the Sync engine so loads never wait on compute engines
    (chunk 0 and the h_last broadcast are split in halves and interleaved
    so compute starts ~7us in).
  - scores for batches 0-3: fused fp32 multiply+row-sum in one DVE
    scalar_tensor_tensor (with the 1/sqrt(D) scale as the immediate);
    batches 4-7: GPSIMD bf16 multiply (stride-2 high-half view of the
    f32 data) + ACT copy reduce with accum_out.  Private h_last copies
    per engine (a shared tile measurably slows both).
  - one ACT relu per chunk writes w into a persistent wall[128,16,8]
    tile; Z comes from one strided reduce + partition_all_reduce at the
    end (no per-chunk accumulate).
  - PE accumulates num[b] into 8 per-batch [1, 512] PSUM banks.
  - epilogue: 1/(Z+eps) on DVE; final scaling split DVE/ACT; two
    strided 8KB stores.
"""

import numpy as np
from contextlib import ExitStack

import concourse.bass as bass
import concourse.tile as tile
from concourse import bacc, mybir
from concourse.bass_utils import run_bass_kernel_spmd

T, B, D = 2048, 64, 512
NCORES = 8
BPC = B // NCORES  # batches per core
P = 128
TC = T // P  # 16 T-chunks
BD = BPC * D  # 4096
SCALE = float(1.0 / np.sqrt(np.float32(D)))
NV = 4  # batches 0..NV-1 on DVE (fused f32 multiply+reduce); rest on GPSIMD
NBUF = 6  # chunk buffers in flight

_nc_cache = None


def _build():
    global _nc_cache
    if _nc_cache is not None:
        return _nc_cache
    nc = bacc.Bacc("TRN2", debug=False, target_bir_lowering=False, num_devices=NCORES)
    h = nc.dram_tensor("h", [T, BPC, D], mybir.dt.float32r, kind="ExternalInput")
    out = nc.dram_tensor("out", [BPC, D], mybir.dt.float32, kind="ExternalOutput")
    h_ap = h.ap()
    out_ap = out.ap()
    f32 = mybir.dt.float32
    f32r = mybir.dt.float32r
    bf16 = mybir.dt.bfloat16

    with tile.TileContext(nc) as tc:
        with ExitStack() as ctx:
            hpool = ctx.enter_context(tc.tile_pool(name="h", bufs=NBUF))
            tmpp = ctx.enter_context(tc.tile_pool(name="tmp", bufs=3))
            tmpg = ctx.enter_context(tc.tile_pool(name="tmpg", bufs=6))
            constp = ctx.enter_context(tc.tile_pool(name="const", bufs=1))
            psp = ctx.enter_context(tc.tile_pool(name="ps", bufs=1, space="PSUM"))

            # h_last broadcast straight from DRAM in two halves so the DVE
            # half is ready early; GPSIMD's half is converted to bf16 (with
            # SCALE folded) by ACT
            hl_f32 = constp.tile([P, NV * D], f32, name="hl_f32")
            hl_gsrc = constp.tile([P, BD - NV * D], f32, name="hl_gsrc")

            def hl_bcast_ap(b0, b1):
                return (
                    h_ap[T - 1 : T, b0:b1, :]
                    .bitcast(f32)
                    .rearrange("p b d -> p (b d)")
                    .broadcast_to([P, (b1 - b0) * D])
                )

            hc_tiles = {}

            def load(c, split=False):
                t = hpool.tile([P, BPC, D], f32r, tag="hc", name="h_sb")
                if split:
                    nc.sync.dma_start(t[:, 0:NV, :], h_ap[c * P : (c + 1) * P, 0:NV, :])
                else:
                    nc.sync.dma_start(t[:], h_ap[c * P : (c + 1) * P, :, :])
                hc_tiles[c] = t

            # startup order: DVE's inputs first, then GPSIMD's
            nc.sync.dma_start(hl_f32[:], hl_bcast_ap(0, NV))
            load(0, split=True)
            nc.sync.dma_start(hl_gsrc[:], hl_bcast_ap(NV, BPC))
            nc.sync.dma_start(
                hc_tiles[0][:, NV:BPC, :], h_ap[0:P, NV:BPC, :]
            )
            for c in range(1, min(NBUF - 1, TC)):
                load(c)

            hl_g = constp.tile([P, BD - NV * D], bf16, name="hl_g")
            nc.scalar.activation(
                hl_g[:], hl_gsrc[:], mybir.ActivationFunctionType.Copy, scale=SCALE
            )
            # DVE's h_last half in bf16 too: the fused multiply reads/writes
            # half the SBUF bytes (in1 and out bf16, in0 stays f32)
            hl_v = constp.tile([P, NV * D], bf16, name="hl_v")
            nc.scalar.activation(
                hl_v[:], hl_f32[:], mybir.ActivationFunctionType.Copy
            )

            # persistent accumulators: one PSUM bank per batch (matmul
            # outputs with K=128 must start at partition 0)
            pouts = [psp.tile([1, D], f32, name=f"pout{b}") for b in range(BPC)]

            # all chunks' scores and relu'd weights live in two small
            # persistent tiles; Z is reduced from wall once at the end
            scrall = constp.tile([P, TC, BPC], f32, name="scrall")
            wall = constp.tile([P, TC, BPC], f32r, name="wall")

            # ACT/DVE writes need quadrant-aligned partition offsets:
            # result rows live at partitions {0,32,64,96} of two tiles.
            res = [constp.tile([P, D], f32, name=f"res{i}") for i in range(2)]
            zsum = constp.tile([P, BPC], f32, name="zsum")
            zred = constp.tile([P, BPC], f32, name="zred")
            zeps = constp.tile([1, BPC], f32, name="zeps")
            zrec = constp.tile([1, BPC], f32, name="zrec")

            for c in range(TC):
                hc = hc_tiles.pop(c)
                scr = scrall[:, c, :]
                w = wall[:, c, :]

                gp_prods = {}
                for b in range(BPC):
                    if b < NV:
                        # fused multiply + row-sum in one DVE pass (f32 h,
                        # bf16 h_last and dummy product output)
                        tmp = tmpp.tile([P, D], bf16, tag="tv")
                        nc.vector.scalar_tensor_tensor(
                            tmp[:],
                            hc[:, b, :].bitcast(f32),
                            SCALE,
                            hl_v[:, b * D : (b + 1) * D],
                            mybir.AluOpType.mult,
                            mybir.AluOpType.mult,
                            accum_out=scr[:, b : b + 1],
                        )
                    else:
                        pt = tmpg.tile([P, D], bf16, tag="tg")
                        gp_prods[b] = pt
                        nc.gpsimd.tensor_tensor(
                            pt[:],
                            hc[:, b, :].bitcast(bf16)[:, 1::2],
                            hl_g[:, (b - NV) * D : (b - NV + 1) * D],
                            mybir.AluOpType.mult,
                        )

                for b, pt in sorted(gp_prods.items()):
                    nc.scalar.activation(
                        pt[:],
                        pt[:],
                        mybir.ActivationFunctionType.Copy,
                        accum_out=scr[:, b : b + 1],
                    )

                nc.scalar.activation(w, scr, mybir.ActivationFunctionType.Relu)

                if c + NBUF - 1 < TC:
                    load(c + NBUF - 1)

                for b in range(BPC):
                    nc.tensor.matmul(
                        pouts[b][:],
                        w[:, b : b + 1],
                        hc[:, b, :],
                        start=(c == 0),
                        stop=(c == TC - 1),
                    )

            # Z[b] = sum over chunks and partitions of relu'd scores
            nc.vector.tensor_reduce(
                zsum[:],
                wall[:].bitcast(f32).rearrange("p c b -> p b c"),
                mybir.AxisListType.X,
                mybir.AluOpType.add,
            )
            nc.gpsimd.partition_all_reduce(
                zred[:], zsum[:], channels=P, reduce_op=bass.bass_isa.ReduceOp.add
            )
            nc.vector.tensor_scalar_add(zeps[:], zred[0:1, :], 1e-9)
            nc.vector.reciprocal(zrec[:], zeps[:])
            for b in range(BPC):
                rt, rrow = res[b // 4], (b % 4) * 32
                if b % 2 == 0:
                    nc.vector.tensor_scalar_mul(
                        rt[rrow : rrow + 1, :], pouts[b][:], zrec[0:1, b : b + 1]
                    )
                else:
                    nc.scalar.mul(
                        rt[rrow : rrow + 1, :], pouts[b][:], zrec[0:1, b : b + 1]
                    )
            nc.sync.dma_start(out_ap[0:4, :], res[0][0:P:32, :])
            nc.sync.dma_start(out_ap[4:8, :], res[1][0:P:32, :])

    nc.finalize()
    _nc_cache = nc
    return nc


def _run(h_all: np.ndarray, trace: bool = False):
    nc = _build()
    h_all = np.ascontiguousarray(np.asarray(h_all), dtype=np.float32)
    assert h_all.shape == (T, B, D)
    in_maps = [
        {"h": np.ascontiguousarray(h_all[:, c * BPC : (c + 1) * BPC, :])}
        for c in range(NCORES)
    ]
    r = run_bass_kernel_spmd(nc, in_maps, list(range(NCORES)), trace=trace)
    out = np.concatenate([r.results[c]["out"] for c in range(NCORES)], axis=0)
    return out, r


def kernel(h_all: np.ndarray, xin: np.ndarray | None = None) -> np.ndarray:
    out, _ = _run(h_all)
    return out


# revision 30
# speedup vs baseline: 1.4274x; 1.0091x over previous
"""AttentivePooling Trainium2 kernel.

Reference semantics (h_all: [T, B, D] f32, xin unused):
    h_last = h_all[-1]                       # [B, D]
    a[b, t] = <h_all[t, b, :], h_last[b, :]> / sqrt(D)
    r = relu(a)
    w = r / (sum_t r + 1e-9)
    out[b, d] = sum_t w[b, t] * h_all[t, b, d]

Strategy: data-parallel over B across 8 cores (8 batches/core, no
collectives).  Per batch on-device (pipelined two batches deep):
  - two 2MB SWDGE DMAs load h_b as 16 SBUF chunks [128(t), 512(d)]
    (t = c*128 + p).  (A single HWDGE dma_start with 2048 descriptors
    wedges the exec unit; SWDGE handles it.)
  - h_last[b] is broadcast across the 128 partitions with a
    partition-stride-0 DMA; ACT mirrors it into PSUM so the DVE
    multiplies read it through the PSUM port (halves SBUF read-port
    pressure; fp32 tensor_tensor is otherwise port-bound at 1x).
  - scores: elementwise multiply split DVE (11 chunks) / GPSIMD (5),
    free-dim reduction split ACT activation-accum (11, with the
    1/sqrt(D) scale folded in) / DVE tensor_reduce (5).  (The fused
    DVE tensor_tensor_reduce crashes the exec unit on this HW.)
  - ACT relu with accum_out produces weights + their per-partition sums
  - PE accumulates sum_t w_t * h_t into PSUM [1, 512] with float32r
    matmuls (w stationary): 1 cycle/row vs fp32's 4, at ~1e-4 rounding
  - PE reduces the weight-sum across partitions via a ones column
  - DVE computes 1/(Z + 1e-9); ACT scales the pooled vector
"""

import numpy as np
from contextlib import ExitStack

import concourse.bass as bass
import concourse.tile as tile
from concourse import bacc, mybir
from concourse.bass_utils import run_bass_kernel_spmd

T, B, D = 2048, 64, 512
NCORES = 8
BPC = B // NCORES  # batches per core
P = 128
TC = T // P  # 16 T-chunks per batch
SCALE = float(1.0 / np.sqrt(np.float32(D)))
DVE_REDUCE_CHUNKS = frozenset({3, 6, 9, 12, 15})  # reduces on DVE; rest on ACT
GP_MULT_CHUNKS = frozenset({2, 4, 7, 9, 12, 14})  # multiplies on GPSIMD; rest on DVE

_nc_cache = None


def _build():
    global _nc_cache
    if _nc_cache is not None:
        return _nc_cache
    nc = bacc.Bacc("TRN2", debug=False, target_bir_lowering=False, num_devices=NCORES)
    h = nc.dram_tensor("h", [T, BPC, D], mybir.dt.float32r, kind="ExternalInput")
    out = nc.dram_tensor("out", [BPC, D], mybir.dt.float32, kind="ExternalOutput")
    h_ap = h.ap()
    out_ap = out.ap()
    f32 = mybir.dt.float32
    f32r = mybir.dt.float32r

    with tile.TileContext(nc) as tc:
        with ExitStack() as ctx:
            hpool = ctx.enter_context(tc.tile_pool(name="h", bufs=5))
            psbcp = ctx.enter_context(tc.tile_pool(name="psb", bufs=2, space="PSUM"))
            tmpp = ctx.enter_context(tc.tile_pool(name="tmp", bufs=6))
            smallp = ctx.enter_context(tc.tile_pool(name="small", bufs=3))
            constp = ctx.enter_context(tc.tile_pool(name="const", bufs=1))
            psoutp = ctx.enter_context(tc.tile_pool(name="pso", bufs=3, space="PSUM"))
            pszp = ctx.enter_context(tc.tile_pool(name="psz", bufs=3, space="PSUM"))

            ones_col = constp.tile([P, 1], f32)
            nc.vector.memset(ones_col[:], 1.0)
            eps_tile = constp.tile([1, 1], f32)
            nc.vector.memset(eps_tile[:], 1e-9)

            def bcast_dma(b):
                # broadcast h_last[b] to all 128 partitions via
                # partition-stride-0 DMA reads straight from DRAM.
                src_bc = h_ap[T - 1 : T, b, :].bitcast(f32).broadcast_to([P, D])
                hlb = smallp.tile([P, D], f32, tag="hlb", name="hlb")
                nc.sync.dma_start(hlb[:], src_bc)
                return hlb

            def bcast_mirror(hlb):
                # ACT mirrors h_last into PSUM (DVE reads it via the PSUM
                # port, halving SBUF read-port pressure; GPSIMD cannot read
                # PSUM so it keeps the SBUF copy).  Runs after the current
                # batch's ACT reduces so it is off the critical path.
                psb = psbcp.tile([P, D], f32, tag="psb")
                nc.scalar.copy(psb[:], hlb[:])
                return psb

            HALF = TC // 2

            def load_h(b):
                t = hpool.tile([P, TC, D], f32r, tag="hsb", name="h_sb")
                src_ap = h_ap[:, b, :].rearrange("(c p) d -> p c d", p=P)
                nc.gpsimd.dma_start(t[:, 0:HALF, :], src_ap[:, 0:HALF, :])
                nc.gpsimd.dma_start(t[:, HALF:TC, :], src_ap[:, HALF:TC, :])
                return t

            h_tiles = {}
            for b in range(min(3, BPC)):
                h_tiles[b] = load_h(b)
            hlb0 = bcast_dma(0)
            psb_tiles = {0: (bcast_mirror(hlb0), hlb0)}
            hlb_tiles = {}

            for b in range(BPC):
                # issue next loads first: the SWDGE descriptor gens then
                # precede this batch's GPSIMD multiplies in program order,
                # so the DMA queues never wait on compute
                if b + 3 < BPC:
                    h_tiles[b + 3] = load_h(b + 3)
                if b + 1 < BPC:
                    hlb_tiles[b + 1] = bcast_dma(b + 1)

                h_sb = h_tiles.pop(b)
                psb, hlb = psb_tiles.pop(b)

                # scores: scr[p, c] = sum_d h[t, d] * hl[d] * SCALE
                scr = smallp.tile([P, TC], f32, tag="scr")
                for c in range(TC):
                    tmp = tmpp.tile([P, D], f32, tag="tmp")
                    if c in GP_MULT_CHUNKS:
                        nc.gpsimd.tensor_tensor(
                            tmp[:],
                            h_sb[:, c, :].bitcast(f32),
                            hlb[:],
                            mybir.AluOpType.mult,
                        )
                    else:
                        nc.vector.tensor_tensor(
                            tmp[:],
                            h_sb[:, c, :].bitcast(f32),
                            psb[:],
                            mybir.AluOpType.mult,
                        )
                    if c in DVE_REDUCE_CHUNKS:
                        nc.vector.tensor_reduce(
                            scr[:, c : c + 1],
                            tmp[:],
                            mybir.AxisListType.X,
                            mybir.AluOpType.add,
                        )
                    else:
                        nc.scalar.activation(
                            tmp[:],
                            tmp[:],
                            mybir.ActivationFunctionType.Copy,
                            scale=SCALE,
                            accum_out=scr[:, c : c + 1],
                        )

                # rescale the DVE-reduced columns (ACT ones had SCALE folded)
                for c in sorted(DVE_REDUCE_CHUNKS):
                    nc.vector.tensor_scalar_mul(
                        scr[:, c : c + 1], scr[:, c : c + 1], SCALE
                    )

                # relu + per-partition sum of relu'd scores
                w = smallp.tile([P, TC], f32r, tag="w")
                zcol = smallp.tile([P, 1], f32, tag="z")
                nc.scalar.activation(
                    w[:], scr[:], mybir.ActivationFunctionType.Relu, accum_out=zcol[:]
                )

                # next batch's PSUM mirror lands after this batch's ACT
                # reduces but before the pooling burst
                if b + 1 < BPC:
                    nhlb = hlb_tiles.pop(b + 1)
                    psb_tiles[b + 1] = (bcast_mirror(nhlb), nhlb)

                # pooled[d] = sum_t w_t * h[t, d] accumulated over chunks
                pout = psoutp.tile([1, D], f32)
                for c in range(TC):
                    nc.tensor.matmul(
                        pout[:],
                        w[:, c : c + 1],
                        h_sb[:, c, :],
                        start=(c == 0),
                        stop=(c == TC - 1),
                    )
                # Z = sum over all t of relu'd scores
                pz = pszp.tile([1, 1], f32)
                nc.tensor.matmul(pz[:], zcol[:], ones_col[:], start=True, stop=True)

                zeps = smallp.tile([1, 1], f32, tag="zeps")
                nc.scalar.activation(
                    zeps[:],
                    pz[:],
                    mybir.ActivationFunctionType.Identity,
                    bias=eps_tile[0:1, 0:1],
                )
                zrec = smallp.tile([1, 1], f32, tag="zrec")
                nc.vector.reciprocal(zrec[:], zeps[:])
                res = smallp.tile([1, D], f32, tag="res")
                nc.scalar.mul(res[:], pout[:], zrec[0:1, 0:1])
                nc.sync.dma_start(out_ap[b : b + 1, :], res[:])

    nc.finalize()
    _nc_cache = nc
    return nc


def _run(h_all: np.ndarray, trace: bool = False):
    nc = _build()
    h_all = np.ascontiguousarray(np.asarray(h_all), dtype=np.float32)
    assert h_all.shape == (T, B, D)
    in_maps = [
        {"h": np.ascontiguousarray(h_all[:, c * BPC : (c + 1) * BPC, :])}
        for c in range(NCORES)
    ]
    r = run_bass_kernel_spmd(nc, in_maps, list(range(NCORES)), trace=trace)
    out = np.concatenate([r.results[c]["out"] for c in range(NCORES)], axis=0)
    return out, r


def kernel(h_all: np.ndarray, xin: np.ndarray | None = None) -> np.ndarray:
    out, _ = _run(h_all)
    return out



# revision 34
# speedup vs baseline: 1.4330x; 1.0039x over previous
"""AttentivePooling Trainium2 kernel.

Reference semantics (h_all: [T, B, D] f32, xin unused):
    h_last = h_all[-1]                       # [B, D]
    a[b, t] = <h_all[t, b, :], h_last[b, :]> / sqrt(D)
    r = relu(a)
    w = r / (sum_t r + 1e-9)
    out[b, d] = sum_t w[b, t] * h_all[t, b, d]

Strategy: data-parallel over B across 8 cores (8 batches/core, no
collectives).  Per batch on-device (pipelined two batches deep):
  - two 2MB SWDGE DMAs load h_b as 16 SBUF chunks [128(t), 512(d)]
    (t = c*128 + p).  (A single HWDGE dma_start with 2048 descriptors
    wedges the exec unit; SWDGE handles it.)
  - h_last[b] is broadcast across the 128 partitions with a
    partition-stride-0 DMA; ACT mirrors it into PSUM so the DVE
    multiplies read it through the PSUM port (halves SBUF read-port
    pressure; fp32 tensor_tensor is otherwise port-bound at 1x).
  - scores: elementwise multiply split DVE (11 chunks) / GPSIMD (5),
    free-dim reduction split ACT activation-accum (11, with the
    1/sqrt(D) scale folded in) / DVE tensor_reduce (5).  (The fused
    DVE tensor_tensor_reduce crashes the exec unit on this HW.)
  - ACT relu with accum_out produces weights + their per-partition sums
  - PE accumulates sum_t w_t * h_t into PSUM [1, 512] with float32r
    matmuls (w stationary): 1 cycle/row vs fp32's 4, at ~1e-4 rounding
  - PE reduces the weight-sum across partitions via a ones column
  - DVE computes 1/(Z + 1e-9); ACT scales the pooled vector
"""

import numpy as np
from contextlib import ExitStack

import concourse.bass as bass
import concourse.tile as tile
from concourse import bacc, mybir
from concourse.bass_utils import run_bass_kernel_spmd

T, B, D = 2048, 64, 512
NCORES = 8
BPC = B // NCORES  # batches per core
P = 128
TC = T // P  # 16 T-chunks per batch
SCALE = float(1.0 / np.sqrt(np.float32(D)))
DVE_REDUCE_CHUNKS = frozenset({3, 6, 9, 12, 15})  # reduces on DVE; rest on ACT
GP_MULT_CHUNKS = frozenset({2, 4, 7, 9, 12, 14})  # multiplies on GPSIMD; rest on DVE

_nc_cache = None


def _build():
    global _nc_cache
    if _nc_cache is not None:
        return _nc_cache
    nc = bacc.Bacc("TRN2", debug=False, target_bir_lowering=False, num_devices=NCORES)
    h = nc.dram_tensor("h", [T, BPC, D], mybir.dt.float32r, kind="ExternalInput")
    out = nc.dram_tensor("out", [BPC, D], mybir.dt.float32, kind="ExternalOutput")
    h_ap = h.ap()
    out_ap = out.ap()
    f32 = mybir.dt.float32
    f32r = mybir.dt.float32r

    with tile.TileContext(nc) as tc:
        with ExitStack() as ctx:
            hpool = ctx.enter_context(tc.tile_pool(name="h", bufs=5))
            psbcp = ctx.enter_context(tc.tile_pool(name="psb", bufs=2, space="PSUM"))
            tmpp = ctx.enter_context(tc.tile_pool(name="tmp", bufs=6))
            smallp = ctx.enter_context(tc.tile_pool(name="small", bufs=3))
            constp = ctx.enter_context(tc.tile_pool(name="const", bufs=1))
            psoutp = ctx.enter_context(tc.tile_pool(name="pso", bufs=3, space="PSUM"))
            pszp = ctx.enter_context(tc.tile_pool(name="psz", bufs=3, space="PSUM"))

            ones_col = constp.tile([P, 1], f32)
            nc.vector.memset(ones_col[:], 1.0)
            eps_tile = constp.tile([1, 1], f32)
            nc.vector.memset(eps_tile[:], 1e-9)

            def bcast_dma(b):
                # broadcast h_last[b] to all 128 partitions via
                # partition-stride-0 DMA reads straight from DRAM.  Issued
                # BEFORE the big loads so its descriptors are not stuck
                # behind megabytes of h traffic in the DMA queues.
                src_bc = h_ap[T - 1 : T, b, :].bitcast(f32).broadcast_to([P, D])
                hlb = smallp.tile([P, D], f32, tag="hlb", name="hlb")
                nc.sync.dma_start(hlb[:], src_bc)
                return hlb

            def bcast_mirror(hlb):
                # ACT mirrors h_last into PSUM (DVE reads it via the PSUM
                # port, halving SBUF read-port pressure; GPSIMD cannot
                # read PSUM so it keeps the SBUF copy).
                psb = psbcp.tile([P, D], f32, tag="psb")
                nc.scalar.copy(psb[:], hlb[:])
                return psb

            HALF = TC // 2

            def load_h(b):
                t = hpool.tile([P, TC, D], f32r, tag="hsb", name="h_sb")
                src_ap = h_ap[:, b, :].rearrange("(c p) d -> p c d", p=P)
                nc.gpsimd.dma_start(t[:, 0:HALF, :], src_ap[:, 0:HALF, :])
                nc.gpsimd.dma_start(t[:, HALF:TC, :], src_ap[:, HALF:TC, :])
                return t

            hlb0 = bcast_dma(0)
            h_tiles = {}
            for b in range(min(2, BPC)):
                h_tiles[b] = load_h(b)
            psb_tiles = {0: (bcast_mirror(hlb0), hlb0)}
            hlb_tiles = {}

            for b in range(BPC):
                # next batch's h_last broadcast goes out first: 128 small
                # descriptors that finish long before they are needed
                if b + 1 < BPC:
                    hlb_tiles[b + 1] = bcast_dma(b + 1)

                h_sb = h_tiles.pop(b)
                psb, hlb = psb_tiles.pop(b)

                # scores: scr[p, c] = sum_d h[t, d] * hl[d] * SCALE
                scr = smallp.tile([P, TC], f32, tag="scr")
                for c in range(TC):
                    if c == 9 and b + 1 < BPC:
                        # mid-stream: mirror the (already landed) next
                        # h_last into PSUM so the next batch's DVE
                        # multiplies start without waiting on this
                        # batch's remaining ACT reduces
                        nhlb = hlb_tiles.pop(b + 1)
                        psb_tiles[b + 1] = (bcast_mirror(nhlb), nhlb)
                    tmp = tmpp.tile([P, D], f32, tag="tmp")
                    if c in GP_MULT_CHUNKS:
                        nc.gpsimd.tensor_tensor(
                            tmp[:],
                            h_sb[:, c, :].bitcast(f32),
                            hlb[:],
                            mybir.AluOpType.mult,
                        )
                    else:
                        nc.vector.tensor_tensor(
                            tmp[:],
                            h_sb[:, c, :].bitcast(f32),
                            psb[:],
                            mybir.AluOpType.mult,
                        )
                    if c in DVE_REDUCE_CHUNKS:
                        nc.vector.tensor_reduce(
                            scr[:, c : c + 1],
                            tmp[:],
                            mybir.AxisListType.X,
                            mybir.AluOpType.add,
                        )
                    else:
                        nc.scalar.activation(
                            tmp[:],
                            tmp[:],
                            mybir.ActivationFunctionType.Copy,
                            scale=SCALE,
                            accum_out=scr[:, c : c + 1],
                        )

                # rescale the DVE-reduced columns (ACT ones had SCALE folded)
                for c in sorted(DVE_REDUCE_CHUNKS):
                    nc.vector.tensor_scalar_mul(
                        scr[:, c : c + 1], scr[:, c : c + 1], SCALE
                    )

                # relu + per-partition sum of relu'd scores
                w = smallp.tile([P, TC], f32r, tag="w")
                zcol = smallp.tile([P, 1], f32, tag="z")
                nc.scalar.activation(
                    w[:], scr[:], mybir.ActivationFunctionType.Relu, accum_out=zcol[:]
                )

                if b + 2 < BPC:
                    h_tiles[b + 2] = load_h(b + 2)

                # pooled[d] = sum_t w_t * h[t, d] accumulated over chunks
                pout = psoutp.tile([1, D], f32)
                for c in range(TC):
                    nc.tensor.matmul(
                        pout[:],
                        w[:, c : c + 1],
                        h_sb[:, c, :],
                        start=(c == 0),
                        stop=(c == TC - 1),
                    )
                # Z = sum over all t of relu'd scores
                pz = pszp.tile([1, 1], f32)
                nc.tensor.matmul(pz[:], zcol[:], ones_col[:], start=True, stop=True)

                zeps = smallp.tile([1, 1], f32, tag="zeps")
                nc.scalar.activation(
                    zeps[:],
                    pz[:],
                    mybir.ActivationFunctionType.Identity,
                    bias=eps_tile[0:1, 0:1],
                )
                zrec = smallp.tile([1, 1], f32, tag="zrec")
                nc.vector.reciprocal(zrec[:], zeps[:])
                res = smallp.tile([1, D], f32, tag="res")
                nc.scalar.mul(res[:], pout[:], zrec[0:1, 0:1])
                nc.sync.dma_start(out_ap[b : b + 1, :], res[:])

    nc.finalize()
    _nc_cache = nc
    return nc


def _run(h_all: np.ndarray, trace: bool = False):
    nc = _build()
    h_all = np.ascontiguousarray(np.asarray(h_all), dtype=np.float32)
    assert h_all.shape == (T, B, D)
    in_maps = [
        {"h": np.ascontiguousarray(h_all[:, c * BPC : (c + 1) * BPC, :])}
        for c in range(NCORES)
    ]
    r = run_bass_kernel_spmd(nc, in_maps, list(range(NCORES)), trace=trace)
    out = np.concatenate([r.results[c]["out"] for c in range(NCORES)], axis=0)
    return out, r


def kernel(h_all: np.ndarray, xin: np.ndarray | None = None) -> np.ndarray:
    out, _ = _run(h_all)
    return out



# revision 35
# speedup vs baseline: 1.4744x; 1.0289x over previous
"""AttentivePooling Trainium2 kernel.

Reference semantics (h_all: [T, B, D] f32, xin unused):
    h_last = h_all[-1]                       # [B, D]
    a[b, t] = <h_all[t, b, :], h_last[b, :]> / sqrt(D)
    r = relu(a)
    w = r / (sum_t r + 1e-9)
    out[b, d] = sum_t w[b, t] * h_all[t, b, d]

Strategy: data-parallel over B across 8 cores (8 batches/core, no
collectives).  Per batch on-device (pipelined two batches deep):
  - two 2MB SWDGE DMAs load h_b as 16 SBUF chunks [128(t), 512(d)]
    (t = c*128 + p).  (A single HWDGE dma_start with 2048 descriptors
    wedges the exec unit; SWDGE handles it.)
  - h_last[b] is broadcast across the 128 partitions with a
    partition-stride-0 DMA; ACT mirrors it into PSUM so the DVE
    multiplies read it through the PSUM port (halves SBUF read-port
    pressure; fp32 tensor_tensor is otherwise port-bound at 1x).
  - scores: elementwise multiply split DVE (11 chunks) / GPSIMD (5),
    free-dim reduction split ACT activation-accum (11, with the
    1/sqrt(D) scale folded in) / DVE tensor_reduce (5).  (The fused
    DVE tensor_tensor_reduce crashes the exec unit on this HW.)
  - ACT relu with accum_out produces weights + their per-partition sums
  - PE accumulates sum_t w_t * h_t into PSUM [1, 512] with float32r
    matmuls (w stationary): 1 cycle/row vs fp32's 4, at ~1e-4 rounding
  - PE reduces the weight-sum across partitions via a ones column
  - DVE computes 1/(Z + 1e-9); ACT scales the pooled vector
"""

import numpy as np
from contextlib import ExitStack

import concourse.bass as bass
import concourse.tile as tile
from concourse import bacc, mybir
from concourse.bass_utils import run_bass_kernel_spmd

T, B, D = 2048, 64, 512
NCORES = 8
BPC = B // NCORES  # batches per core
P = 128
TC = T // P  # 16 T-chunks per batch
SCALE = float(1.0 / np.sqrt(np.float32(D)))
DVE_REDUCE_CHUNKS = frozenset({3, 6, 9, 12, 15})  # reduces on DVE; rest on ACT
GP_MULT_CHUNKS = frozenset({2, 4, 7, 9, 12, 14})  # multiplies on GPSIMD; rest on DVE

_nc_cache = None


def _build():
    global _nc_cache
    if _nc_cache is not None:
        return _nc_cache
    nc = bacc.Bacc("TRN2", debug=False, target_bir_lowering=False, num_devices=NCORES)
    h = nc.dram_tensor("h", [T, BPC, D], mybir.dt.float32r, kind="ExternalInput")
    out = nc.dram_tensor("out", [BPC, D], mybir.dt.float32, kind="ExternalOutput")
    h_ap = h.ap()
    out_ap = out.ap()
    f32 = mybir.dt.float32
    f32r = mybir.dt.float32r

    with tile.TileContext(nc) as tc:
        with ExitStack() as ctx:
            hpool = ctx.enter_context(tc.tile_pool(name="h", bufs=5))
            psbcp = ctx.enter_context(tc.tile_pool(name="psb", bufs=2, space="PSUM"))
            tmpp = ctx.enter_context(tc.tile_pool(name="tmp", bufs=6))
            smallp = ctx.enter_context(tc.tile_pool(name="small", bufs=3))
            constp = ctx.enter_context(tc.tile_pool(name="const", bufs=1))
            psoutp = ctx.enter_context(tc.tile_pool(name="pso", bufs=3, space="PSUM"))
            pszp = ctx.enter_context(tc.tile_pool(name="psz", bufs=3, space="PSUM"))

            ones_col = constp.tile([P, 1], f32)
            nc.vector.memset(ones_col[:], 1.0)
            eps_tile = constp.tile([1, 1], f32)
            nc.vector.memset(eps_tile[:], 1e-9)

            def bcast(b):
                # broadcast h_last[b] to all 128 partitions via
                # partition-stride-0 DMA reads straight from DRAM.
                # One copy lands in PSUM (DVE reads it via the PSUM port,
                # halving SBUF read-port pressure), one in SBUF (GPSIMD
                # cannot read PSUM).
                src_bc = h_ap[T - 1 : T, b, :].bitcast(f32).broadcast_to([P, D])
                hlb = smallp.tile([P, D], f32, tag="hlb", name="hlb")
                nc.sync.dma_start(hlb[:], src_bc)
                psb = psbcp.tile([P, D], f32, tag="psb")
                nc.scalar.copy(psb[:], hlb[:])
                return psb, hlb

            HALF = TC // 2

            def load_h(b):
                t = hpool.tile([P, TC, D], f32r, tag="hsb", name="h_sb")
                src_ap = h_ap[:, b, :].rearrange("(c p) d -> p c d", p=P)
                nc.gpsimd.dma_start(t[:, 0:HALF, :], src_ap[:, 0:HALF, :])
                nc.gpsimd.dma_start(t[:, HALF:TC, :], src_ap[:, HALF:TC, :])
                return t

            h_tiles = {}
            for b in range(min(2, BPC)):
                h_tiles[b] = load_h(b)
            psb_tiles = {0: bcast(0)}

            for b in range(BPC):
                h_sb = h_tiles.pop(b)
                psb, hlb = psb_tiles.pop(b)

                # scores: scr[p, c] = sum_d h[t, d] * hl[d] * SCALE
                scr = smallp.tile([P, TC], f32, tag="scr")
                for c in range(TC):
                    tmp = tmpp.tile([P, D], f32, tag="tmp")
                    if c in GP_MULT_CHUNKS:
                        nc.gpsimd.tensor_tensor(
                            tmp[:],
                            h_sb[:, c, :].bitcast(f32),
                            hlb[:],
                            mybir.AluOpType.mult,
                        )
                    else:
                        nc.vector.tensor_tensor(
                            tmp[:],
                            h_sb[:, c, :].bitcast(f32),
                            psb[:],
                            mybir.AluOpType.mult,
                        )
                    if c in DVE_REDUCE_CHUNKS:
                        nc.vector.tensor_reduce(
                            scr[:, c : c + 1],
                            tmp[:],
                            mybir.AxisListType.X,
                            mybir.AluOpType.add,
                        )
                    else:
                        nc.scalar.activation(
                            tmp[:],
                            tmp[:],
                            mybir.ActivationFunctionType.Copy,
                            scale=SCALE,
                            accum_out=scr[:, c : c + 1],
                        )

                # rescale the DVE-reduced columns (ACT ones had SCALE folded)
                for c in sorted(DVE_REDUCE_CHUNKS):
                    nc.vector.tensor_scalar_mul(
                        scr[:, c : c + 1], scr[:, c : c + 1], SCALE
                    )

                # relu + per-partition sum of relu'd scores
                w = smallp.tile([P, TC], f32r, tag="w")
                zcol = smallp.tile([P, 1], f32, tag="z")
                nc.scalar.activation(
                    w[:], scr[:], mybir.ActivationFunctionType.Relu, accum_out=zcol[:]
                )

                # next batch's broadcast goes to PE BEFORE this batch's
                # pooling burst, so the next scores phase is not blocked
                # behind the pooling in PE program order
                if b + 1 < BPC:
                    psb_tiles[b + 1] = bcast(b + 1)
                if b + 2 < BPC:
                    h_tiles[b + 2] = load_h(b + 2)

                # pooled[d] = sum_t w_t * h[t, d] accumulated over chunks
                pout = psoutp.tile([1, D], f32)
                for c in range(TC):
                    nc.tensor.matmul(
                        pout[:],
                        w[:, c : c + 1],
                        h_sb[:, c, :],
                        start=(c == 0),
                        stop=(c == TC - 1),
                    )
                # Z = sum over all t of relu'd scores
                pz = pszp.tile([1, 1], f32)
                nc.tensor.matmul(pz[:], zcol[:], ones_col[:], start=True, stop=True)

                zeps = smallp.tile([1, 1], f32, tag="zeps")
                nc.scalar.activation(
                    zeps[:],
                    pz[:],
                    mybir.ActivationFunctionType.Identity,
                    bias=eps_tile[0:1, 0:1],
                )
                zrec = smallp.tile([1, 1], f32, tag="zrec")
                nc.vector.reciprocal(zrec[:], zeps[:])
                res = smallp.tile([1, D], f32, tag="res")
                nc.scalar.mul(res[:], pout[:], zrec[0:1, 0:1])
                nc.sync.dma_start(out_ap[b : b + 1, :], res[:])

    nc.finalize()
    _nc_cache = nc
    return nc


def _run(h_all: np.ndarray, trace: bool = False):
    nc = _build()
    h_all = np.ascontiguousarray(np.asarray(h_all), dtype=np.float32)
    assert h_all.shape == (T, B, D)
    in_maps = [
        {"h": np.ascontiguousarray(h_all[:, c * BPC : (c + 1) * BPC, :])}
        for c in range(NCORES)
    ]
    r = run_bass_kernel_spmd(nc, in_maps, list(range(NCORES)), trace=trace)
    out = np.concatenate([r.results[c]["out"] for c in range(NCORES)], axis=0)
    return out, r


def kernel(h_all: np.ndarray, xin: np.ndarray | None = None) -> np.ndarray:
    out, _ = _run(h_all)
    return out



# revision 36
# speedup vs baseline: 1.5123x; 1.0257x over previous
"""AttentivePooling Trainium2 kernel.

Reference semantics (h_all: [T, B, D] f32, xin unused):
    h_last = h_all[-1]                       # [B, D]
    a[b, t] = <h_all[t, b, :], h_last[b, :]> / sqrt(D)
    r = relu(a)
    w = r / (sum_t r + 1e-9)
    out[b, d] = sum_t w[b, t] * h_all[t, b, d]

Strategy: data-parallel over B across 8 cores (8 batches/core, no
collectives).  Per batch on-device (pipelined two batches deep):
  - two 2MB SWDGE DMAs load h_b as 16 SBUF chunks [128(t), 512(d)]
    (t = c*128 + p).  (A single HWDGE dma_start with 2048 descriptors
    wedges the exec unit; SWDGE handles it.)
  - h_last[b] is broadcast across the 128 partitions with a
    partition-stride-0 DMA; ACT mirrors it into PSUM so the DVE
    multiplies read it through the PSUM port (halves SBUF read-port
    pressure; fp32 tensor_tensor is otherwise port-bound at 1x).
  - scores: elementwise multiply split DVE (11 chunks) / GPSIMD (5),
    free-dim reduction split ACT activation-accum (11, with the
    1/sqrt(D) scale folded in) / DVE tensor_reduce (5).  (The fused
    DVE tensor_tensor_reduce crashes the exec unit on this HW.)
  - ACT relu with accum_out produces weights + their per-partition sums
  - PE accumulates sum_t w_t * h_t into PSUM [1, 512] with float32r
    matmuls (w stationary): 1 cycle/row vs fp32's 4, at ~1e-4 rounding
  - PE reduces the weight-sum across partitions via a ones column
  - DVE computes 1/(Z + 1e-9); ACT scales the pooled vector
"""

import numpy as np
from contextlib import ExitStack

import concourse.bass as bass
import concourse.tile as tile
from concourse import bacc, mybir
from concourse.bass_utils import run_bass_kernel_spmd

T, B, D = 2048, 64, 512
NCORES = 8
BPC = B // NCORES  # batches per core
P = 128
TC = T // P  # 16 T-chunks per batch
SCALE = float(1.0 / np.sqrt(np.float32(D)))
DVE_REDUCE_CHUNKS = frozenset({3, 6, 9, 12, 15})  # reduces on DVE; rest on ACT
GP_MULT_CHUNKS = frozenset({2, 4, 7, 9, 12, 14})  # multiplies on GPSIMD; rest on DVE

_nc_cache = None


def _build():
    global _nc_cache
    if _nc_cache is not None:
        return _nc_cache
    nc = bacc.Bacc("TRN2", debug=False, target_bir_lowering=False, num_devices=NCORES)
    h = nc.dram_tensor("h", [T, BPC, D], mybir.dt.float32r, kind="ExternalInput")
    out = nc.dram_tensor("out", [BPC, D], mybir.dt.float32, kind="ExternalOutput")
    h_ap = h.ap()
    out_ap = out.ap()
    f32 = mybir.dt.float32
    f32r = mybir.dt.float32r

    with tile.TileContext(nc) as tc:
        with ExitStack() as ctx:
            hpool = ctx.enter_context(tc.tile_pool(name="h", bufs=5))
            psbcp = ctx.enter_context(tc.tile_pool(name="psb", bufs=2, space="PSUM"))
            tmpp = ctx.enter_context(tc.tile_pool(name="tmp", bufs=6))
            smallp = ctx.enter_context(tc.tile_pool(name="small", bufs=3))
            constp = ctx.enter_context(tc.tile_pool(name="const", bufs=1))
            psoutp = ctx.enter_context(tc.tile_pool(name="pso", bufs=3, space="PSUM"))
            pszp = ctx.enter_context(tc.tile_pool(name="psz", bufs=3, space="PSUM"))

            ones_col = constp.tile([P, 1], f32)
            nc.vector.memset(ones_col[:], 1.0)
            eps_tile = constp.tile([1, 1], f32)
            nc.vector.memset(eps_tile[:], 1e-9)

            def bcast(b):
                # broadcast h_last[b] to all 128 partitions via
                # partition-stride-0 DMA reads straight from DRAM.
                # One copy lands in PSUM (DVE reads it via the PSUM port,
                # halving SBUF read-port pressure), one in SBUF (GPSIMD
                # cannot read PSUM).
                src_bc = h_ap[T - 1 : T, b, :].bitcast(f32).broadcast_to([P, D])
                hlb = smallp.tile([P, D], f32, tag="hlb", name="hlb")
                nc.sync.dma_start(hlb[:], src_bc)
                psb = psbcp.tile([P, D], f32, tag="psb")
                nc.scalar.copy(psb[:], hlb[:])
                return psb, hlb

            HALF = TC // 2

            def load_h(b):
                t = hpool.tile([P, TC, D], f32r, tag="hsb", name="h_sb")
                src_ap = h_ap[:, b, :].rearrange("(c p) d -> p c d", p=P)
                nc.gpsimd.dma_start(t[:, 0:HALF, :], src_ap[:, 0:HALF, :])
                nc.gpsimd.dma_start(t[:, HALF:TC, :], src_ap[:, HALF:TC, :])
                return t

            # batch 0's h_last broadcast goes out before the big loads:
            # its 128 small descriptors land in ~1us instead of queueing
            # behind 4MB of h traffic, so batch 0's DVE multiplies start
            # as soon as the first half-load completes
            psb_tiles = {0: bcast(0)}
            h_tiles = {}
            for b in range(min(2, BPC)):
                h_tiles[b] = load_h(b)

            for b in range(BPC):
                h_sb = h_tiles.pop(b)
                psb, hlb = psb_tiles.pop(b)

                # scores: scr[p, c] = sum_d h[t, d] * hl[d] * SCALE
                scr = smallp.tile([P, TC], f32, tag="scr")
                for c in range(TC):
                    tmp = tmpp.tile([P, D], f32, tag="tmp")
                    if c in GP_MULT_CHUNKS:
                        nc.gpsimd.tensor_tensor(
                            tmp[:],
                            h_sb[:, c, :].bitcast(f32),
                            hlb[:],
                            mybir.AluOpType.mult,
                        )
                    else:
                        nc.vector.tensor_tensor(
                            tmp[:],
                            h_sb[:, c, :].bitcast(f32),
                            psb[:],
                            mybir.AluOpType.mult,
                        )
                    if c in DVE_REDUCE_CHUNKS:
                        nc.vector.tensor_reduce(
                            scr[:, c : c + 1],
                            tmp[:],
                            mybir.AxisListType.X,
                            mybir.AluOpType.add,
                        )
                    else:
                        nc.scalar.activation(
                            tmp[:],
                            tmp[:],
                            mybir.ActivationFunctionType.Copy,
                            scale=SCALE,
                            accum_out=scr[:, c : c + 1],
                        )

                # rescale the DVE-reduced columns (ACT ones had SCALE folded)
                for c in sorted(DVE_REDUCE_CHUNKS):
                    nc.vector.tensor_scalar_mul(
                        scr[:, c : c + 1], scr[:, c : c + 1], SCALE
                    )

                # relu + per-partition sum of relu'd scores
                w = smallp.tile([P, TC], f32r, tag="w")
                zcol = smallp.tile([P, 1], f32, tag="z")
                nc.scalar.activation(
                    w[:], scr[:], mybir.ActivationFunctionType.Relu, accum_out=zcol[:]
                )

                # next batch's broadcast goes to PE BEFORE this batch's
                # pooling burst, so the next scores phase is not blocked
                # behind the pooling in PE program order
                if b + 1 < BPC:
                    psb_tiles[b + 1] = bcast(b + 1)
                if b + 2 < BPC:
                    h_tiles[b + 2] = load_h(b + 2)

                # pooled[d] = sum_t w_t * h[t, d] accumulated over chunks
                pout = psoutp.tile([1, D], f32)
                for c in range(TC):
                    nc.tensor.matmul(
                        pout[:],
                        w[:, c : c + 1],
                        h_sb[:, c, :],
                        start=(c == 0),
                        stop=(c == TC - 1),
                    )
                # Z = sum over all t of relu'd scores
                pz = pszp.tile([1, 1], f32)
                nc.tensor.matmul(pz[:], zcol[:], ones_col[:], start=True, stop=True)

                zeps = smallp.tile([1, 1], f32, tag="zeps")
                nc.scalar.activation(
                    zeps[:],
                    pz[:],
                    mybir.ActivationFunctionType.Identity,
                    bias=eps_tile[0:1, 0:1],
                )
                zrec = smallp.tile([1, 1], f32, tag="zrec")
                nc.vector.reciprocal(zrec[:], zeps[:])
                res = smallp.tile([1, D], f32, tag="res")
                nc.scalar.mul(res[:], pout[:], zrec[0:1, 0:1])
                nc.sync.dma_start(out_ap[b : b + 1, :], res[:])

    nc.finalize()
    _nc_cache = nc
    return nc


def _run(h_all: np.ndarray, trace: bool = False):
    nc = _build()
    h_all = np.ascontiguousarray(np.asarray(h_all), dtype=np.float32)
    assert h_all.shape == (T, B, D)
    in_maps = [
        {"h": np.ascontiguousarray(h_all[:, c * BPC : (c + 1) * BPC, :])}
        for c in range(NCORES)
    ]
    r = run_bass_kernel_spmd(nc, in_maps, list(range(NCORES)), trace=trace)
    out = np.concatenate([r.results[c]["out"] for c in range(NCORES)], axis=0)
    return out, r


def kernel(h_all: np.ndarray, xin: np.ndarray | None = None) -> np.ndarray:
    out, _ = _run(h_all)
    return out

